# revision 42
# baseline (speedup 1.0000x reference)
"""Cosine-similarity multi-head attention on 8 Trainium2 NeuronCores.

Sharding: data/sequence-parallel. Core c (c = b*4 + qs) computes the full
output rows for query tokens [qs*512, (qs+1)*512) of batch b.  Each core
computes K and V for its whole batch (duplicated 4x across the cores sharing
the batch; collectives measured too slow to be worth deduplicating).

Design notes (v3):
  - Whole attention path in bf16 (qnT/knf/et/av/onT/wo).
  - K projection is *lazy*: chunk f (the feature rows of head pair 2f,2f+1)
    is projected + normalized one f ahead, interleaved into phase B's S/AV
    slots, keeping the PE busy while the ACT engine streams softmax exps
    (16.8M elements at 1 elem/cycle = the phase-B floor).  kproj(0) is
    interleaved into the tail of phase A's V projection the same way.
  - All norm rsqrts via a DVE bit-trick (+2 Newton steps); the tiny
    per-block |k|^2 matmuls are col-tiled (tile_position=(0,32t)) into one
    PSUM bank so one [128,512] DVE chain covers a whole head pair.
  - Output projection is split: g=0..3 (heads normalized by the first
    denominator batch) runs interleaved into f=7's slots and is stashed to
    SBUF; g=4..7 runs after the last denominator batch.  This fills the
    phase-B tail and removes the B->C bubble.
  - softmax uses no max-subtraction: |logits| <= scale = 10, exp safe f32.
"""

import numpy as np

B, N, DIM, H, DH = 2, 2048, 1024, 16, 64
INNER = H * DH
NQ = 512            # query tokens per core
P = 128
KC = DIM // P       # 8 feature chunks of 128
JC = N // P         # 16 key-token chunks of 128
NB = N // NQ        # 4 token blocks of 512
MAX_LOG_SCALE = float(np.log(1.0 / 0.01))
MAGIC = 0x5F3759DF

_CACHE = {}


def _build():
    if "nc" in _CACHE:
        return _CACHE["nc"]
    import concourse.bass as bass
    import concourse.bacc as bacc
    import concourse.mybir as mybir
    import concourse.tile as tile

    f32 = mybir.dt.float32
    i32 = mybir.dt.int32
    bf16 = mybir.dt.bfloat16
    AF = mybir.ActivationFunctionType
    ALU = mybir.AluOpType

    nc = bacc.Bacc("TRN2", target_bir_lowering=False)

    xq_in = nc.declare_dram_parameter("xq", [P, KC, NQ], bf16, isOutput=False)
    xTb = nc.declare_dram_parameter("xTb", [P, KC, N], bf16, isOutput=False)
    wqb = nc.declare_dram_parameter("wqb", [P, KC, KC, P], bf16, isOutput=False)
    wkb = nc.declare_dram_parameter("wkb", [P, KC, KC, P], bf16, isOutput=False)
    wvb = nc.declare_dram_parameter("wvb", [P, 2, KC, INNER // 2], bf16, isOutput=False)
    wo2 = nc.declare_dram_parameter("wo2", [P, KC, KC, P], bf16, isOutput=False)
    bout = nc.declare_dram_parameter("bout", [P, KC], f32, isOutput=False)
    # scale_h spread to rows 32*(m%4)+half, col = bank m//4 (norm-matmul layout)
    sclb = nc.declare_dram_parameter("sclb", [P, 2], f32, isOutput=False)
    outT = nc.declare_dram_parameter("outT", [DIM, NQ], f32, isOutput=True)

    # internal DRAM scratch (broadcast round trips)
    fq_d = nc.dram_tensor("fq_d", [2, P, NQ], f32)
    fk_d = nc.dram_tensor("fk_d", [KC, P, NQ], bf16)
    dnf_d = nc.dram_tensor("dnf_d", [H, NQ], f32)
    dnm_d = nc.dram_tensor("dnm_d", [H, NQ], bf16)

    def rsqrt_dve(y, x, u, out=None):
        """y/out = 1/sqrt(x) elementwise on DVE (bit-trick + 2 Newton).
        x,y,u f32 APs (same shape); if out given the final step writes it."""
        nc.vector.tensor_scalar(
            out=y.bitcast(i32), in0=x.bitcast(i32),
            scalar1=1, scalar2=None, op0=ALU.arith_shift_right)
        nc.vector.tensor_scalar(
            out=y.bitcast(i32), in0=y.bitcast(i32),
            scalar1=-1, scalar2=MAGIC, op0=ALU.mult, op1=ALU.add)
        for it in range(2):
            nc.vector.tensor_mul(u, y, y)
            nc.vector.tensor_mul(u, u, x)
            nc.vector.tensor_scalar(
                out=u, in0=u, scalar1=-0.5, scalar2=1.5,
                op0=ALU.mult, op1=ALU.add)
            dst = y if (it == 0 or out is None) else out
            nc.vector.tensor_mul(dst, y, u)

    with tile.TileContext(nc) as tc:
        with (
            tc.tile_pool(name="persist", bufs=1) as pp,
            tc.tile_pool(name="pBk", bufs=2) as pbk,
            tc.tile_pool(name="pBs", bufs=2) as pbs,
            tc.tile_pool(name="psK", bufs=1, space="PSUM") as psK,
            tc.tile_pool(name="psNK", bufs=1, space="PSUM") as psNK,
        ):
            xq = pp.tile([P, KC, NQ], bf16, tag="xq")
            xt = pp.tile([P, KC, N], bf16, tag="xt")
            knf = pp.tile([P, KC, N], bf16, tag="knf")
            av = pp.tile([P, JC, H, DH + 1], bf16, tag="av")
            qnT = pp.tile([P, KC, NQ], bf16, tag="qnT")
            onTa = pp.tile([P, KC // 2, NQ], bf16, tag="onTa")
            onTb = pp.tile([P, KC // 2, NQ], bf16, tag="onTb")
            wo_sb = pp.tile([P, KC, KC, P], bf16, tag="wo")
            scale_sb = pp.tile([P, 2], f32, tag="scale")
            bout_sb = pp.tile([P, KC], f32, tag="bout")
            hsm2 = pp.tile([P, 2], bf16, tag="hsm2")

            def onT(g):
                return onTa[:, g, :] if g < KC // 2 else onTb[:, g - KC // 2, :]

            # queue layout: sync = xq + per-m Q weights + broadcasts;
            # gpsimd = xt + wo; vector = wvb; scalar = small consts
            nc.sync.dma_start(out=xq[:], in_=xq_in[:])
            nc.gpsimd.dma_start(out=xt[:], in_=xTb[:])
            nc.scalar.dma_start(out=wo_sb[:], in_=wo2[:])
            nc.scalar.dma_start(out=scale_sb[:], in_=sclb[:])
            nc.scalar.dma_start(out=bout_sb[:], in_=bout[:])
            nc.vector.memset(hsm2[:], 0.0)
            nc.vector.memset(hsm2[0:64, 0:1], 1.0)
            nc.vector.memset(hsm2[64:P, 1:2], 1.0)
            nc.vector.memset(av[:, :, :, DH], 1.0)

            def kproj_gen(f, src, wq=None):
                """Project + normalize K chunk f into knf[:, f, :].
                Yields 8x (2 per token block); norm chain emitted at drain."""
                wt = pbk.tile([P, KC, P], bf16, tag="wk", name=f"wk{f}")
                (wq or nc.sync).dma_start(out=wt[:], in_=wkb[:, f])
                nkb = psNK.tile([P, NQ], f32, tag="nkb", name=f"nkb{f}")
                for t in range(NB):
                    tsl = slice(t * NQ, (t + 1) * NQ)
                    ps = psK.tile([P, NQ], f32, tag="kp", name=f"kps{f}_{t}")
                    for kc in range(4):
                        nc.tensor.matmul(ps[:], wt[:, kc, :], src[:, kc, tsl],
                                         start=(kc == 0), stop=False)
                    yield
                    for kc in range(4, KC):
                        nc.tensor.matmul(ps[:], wt[:, kc, :], src[:, kc, tsl],
                                         start=False, stop=(kc == KC - 1))
                    nc.vector.tensor_copy(knf[:, f, tsl], ps[:])
                    sq = pbs.tile([P, NQ], bf16, tag="ksq")
                    nc.gpsimd.tensor_mul(sq[:], knf[:, f, tsl], knf[:, f, tsl])
                    co = 32 * t
                    nc.tensor.matmul(nkb[co:co + 2, :], hsm2[:], sq[:],
                                     start=True, stop=True,
                                     tile_position=(0, co))
                    yield
                # 1/|k| and fold into knf; rows 32t+half of the norm bank
                # carry (block t, head 2f+half), other rows are junk
                fkx = pbs.tile([P, NQ], f32, tag="fkx")
                nc.vector.tensor_copy(fkx[:], nkb[:])
                fku = pbs.tile([P, NQ], f32, tag="fku")
                fkv = pbs.tile([P, NQ], f32, tag="fkv")
                fk16 = pbs.tile([P, NQ], bf16, tag="fk16")
                rsqrt_dve(fku[:], fkx[:], fkv[:], out=fk16[:])
                nc.sync.dma_start(out=fk_d[f], in_=fk16[:])
                for t in range(NB):
                    tsl = slice(t * NQ, (t + 1) * NQ)
                    ro = 32 * t
                    fkb = pbs.tile([P, NQ], bf16, tag="fkb")
                    nc.sync.dma_start(
                        out=fkb[0:64, :],
                        in_=fk_d[f, ro:ro + 1, :].to_broadcast((64, NQ)))
                    nc.sync.dma_start(
                        out=fkb[64:P, :],
                        in_=fk_d[f, ro + 1:ro + 2, :].to_broadcast((64, NQ)))
                    nc.vector.tensor_mul(knf[:, f, tsl], knf[:, f, tsl], fkb[:])

            # ---------------- Phase A: Q + V projections ----------------
            with (
                tc.tile_pool(name="pA", bufs=2) as pa,
                tc.tile_pool(name="pAv", bufs=1) as pav,
                tc.tile_pool(name="pAs", bufs=2) as pas,
                tc.tile_pool(name="pAq", bufs=1) as paq,
                tc.tile_pool(name="psA", bufs=2, space="PSUM") as psA,
                tc.tile_pool(name="psN", bufs=2, space="PSUM") as psN,
            ):
                qraw = paq.tile([P, KC, NQ], f32, tag="qraw")

                nqb = [psN.tile([P, NQ], f32, tag="nqb", name=f"nqb{bk}")
                       for bk in range(2)]
                for m in range(KC):
                    wt = pa.tile([P, KC, P], bf16, tag="w")
                    nc.sync.dma_start(out=wt[:], in_=wqb[:, m])
                    ps = psA.tile([P, NQ], f32, tag="qp")
                    for kc in range(KC):
                        nc.tensor.matmul(ps[:], wt[:, kc, :], xq[:, kc, :],
                                         start=(kc == 0), stop=(kc == KC - 1))
                    nc.scalar.copy(qraw[:, m, :], ps[:])
                    sq = pas.tile([P, NQ], bf16, tag="sq")
                    nc.vector.tensor_mul(sq[:], qraw[:, m, :], qraw[:, m, :])
                    co = 32 * (m % 4)
                    nc.tensor.matmul(nqb[m // 4][co:co + 2, :], hsm2[:], sq[:],
                                     start=True, stop=True,
                                     tile_position=(0, co))

                # fq = scale_h / |q|  (rows 32*(m%4)+half carry head 2m+half)
                for bk in range(2):
                    fqx = pas.tile([P, NQ], f32, tag="fqx")
                    nc.vector.tensor_copy(fqx[:], nqb[bk][:])
                    fqy = pas.tile([P, NQ], f32, tag="fqy")
                    fqu = pas.tile([P, NQ], f32, tag="fqu")
                    rsqrt_dve(fqy[:], fqx[:], fqu[:])
                    nc.vector.tensor_scalar_mul(fqy[:], fqy[:],
                                                scale_sb[:, bk:bk + 1])
                    nc.sync.dma_start(out=fq_d[bk], in_=fqy[:])
                for m in range(KC):
                    ro = 32 * (m % 4)
                    fqb = pas.tile([P, NQ], f32, tag="fqb")
                    nc.sync.dma_start(
                        out=fqb[0:64, :],
                        in_=fq_d[m // 4, ro:ro + 1, :].to_broadcast((64, NQ)))
                    nc.sync.dma_start(
                        out=fqb[64:P, :],
                        in_=fq_d[m // 4, ro + 1:ro + 2, :].to_broadcast((64, NQ)))
                    nc.vector.tensor_mul(qnT[:, m, :], qraw[:, m, :], fqb[:])

                # V projection (token-major into av, bf16), with K chunk 0
                # interleaved into the second half
                k0 = kproj_gen(0, xt)
                for fb in range(2):
                    wv = pav.tile([P, KC, INNER // 2], bf16, tag="wv")
                    nc.gpsimd.dma_start(out=wv[:], in_=wvb[:, fb])
                    for jc in range(JC):
                        ps = psA.tile([P, NQ], f32, tag="vp")
                        for kc in range(KC):
                            nc.tensor.matmul(ps[:], xt[:, kc, jc * P:(jc + 1) * P],
                                             wv[:, kc, :],
                                             start=(kc == 0), stop=(kc == KC - 1))
                        nc.vector.tensor_copy(
                            av[:, jc, fb * 8:(fb + 1) * 8, 0:DH],
                            ps[:].rearrange("p (h d) -> p h d", d=DH))
                        if fb == 1:
                            if 4 <= jc < 8:
                                next(k0, None)
                                next(k0, None)
                            elif jc == 8:
                                for _ in k0:
                                    pass

            # ---------------- Phase B: lazy K proj + attention ----------------
            with (
                tc.tile_pool(name="pBe", bufs=3) as pbe,
                tc.tile_pool(name="pBr", bufs=2) as pbr,
                tc.tile_pool(name="pC", bufs=1) as pc,
                tc.tile_pool(name="pC1", bufs=1) as pc1,
                tc.tile_pool(name="psS", bufs=2, space="PSUM") as psS,
                tc.tile_pool(name="psV", bufs=1, space="PSUM") as psV,
            ):
                oc1 = pc1.tile([P, KC, NQ], f32, tag="oc1")

                def emit_av(f, jc2, ets, avps):
                    for q in range(2):
                        jc = 2 * jc2 + q
                        for half in range(2):
                            h = 2 * f + half
                            nc.tensor.matmul(
                                avps[half][0:DH + 1, :],
                                av[:, jc, h, :], ets[(jc2, half)][:, q, :],
                                start=(jc == 0), stop=(jc == JC - 1))

                def attn_f(f, ksteps):
                    avps = [psV.tile([P, NQ], f32, tag=f"avp{half}",
                                     name=f"avp{half}_{f}")
                            for half in range(2)]
                    ets = {}
                    for jc2 in range(8):
                        sp2 = []
                        for half in range(2):
                            sp2.append(psS.tile([P, 2, NQ], f32,
                                                tag="sps",
                                                name=f"sps{half}_{f}_{jc2}"))
                        for q in range(2):
                            jc = 2 * jc2 + q
                            for half in range(2):
                                lo = 64 * half
                                nc.tensor.matmul(
                                    sp2[half][:, q, :],
                                    knf[lo:lo + 64, f, jc * P:(jc + 1) * P],
                                    qnT[lo:lo + 64, f, :],
                                    start=True, stop=True, tile_position=(lo, 0))
                        for half in range(2):
                            et = pbe.tile([P, 2, NQ], bf16, tag=f"et{half}")
                            nc.scalar.activation(et[:], sp2[half][:], AF.Exp)
                            ets[(jc2, half)] = et
                        if ksteps is not None:
                            if jc2 < 4:
                                next(ksteps, None)
                                next(ksteps, None)
                            elif jc2 == 4:
                                for _ in ksteps:
                                    pass
                        if jc2 > 0:
                            emit_av(f, jc2 - 1, ets, avps)
                    emit_av(f, 7, ets, avps)
                    # attention outputs + denominators for this head pair
                    for half in range(2):
                        h = 2 * f + half
                        lo = 64 * half
                        nc.vector.tensor_copy(onT(f)[lo:lo + 64, :],
                                              avps[half][0:DH, :])
                        dcp = pbr.tile([1, NQ], f32, tag="dcp")
                        nc.vector.tensor_copy(dcp[:], avps[half][DH:DH + 1, :])
                        nc.sync.dma_start(out=dnf_d[h:h + 1, :], in_=dcp[:])
                    # normalize finished head pairs in two batches
                    if f in (KC // 2 - 1, KC - 1):
                        hlo = 0 if f == KC // 2 - 1 else H // 2
                        dnm = pbr.tile([H // 2, NQ], f32, tag="dnm")
                        nc.sync.dma_start(out=dnm[:], in_=dnf_d[hlo:hlo + 8, :])
                        nc.vector.reciprocal(dnm[:], dnm[:])
                        dnr = pbr.tile([H // 2, NQ], bf16, tag="dnr")
                        nc.vector.tensor_copy(dnr[:], dnm[:])
                        nc.sync.dma_start(out=dnm_d[hlo:hlo + 8, :], in_=dnr[:])
                        for ff in range(hlo // 2, hlo // 2 + 4):
                            g = ff if hlo == 0 else ff
                            dnb = pbr.tile([P, NQ], bf16, tag="dnb")
                            nc.sync.dma_start(
                                out=dnb[0:64, :],
                                in_=dnm_d[2 * ff:2 * ff + 1, :]
                                .to_broadcast((64, NQ)))
                            nc.sync.dma_start(
                                out=dnb[64:P, :],
                                in_=dnm_d[2 * ff + 1:2 * ff + 2, :]
                                .to_broadcast((64, NQ)))
                            nc.vector.tensor_mul(onT(ff), onT(ff), dnb[:])

                def phasec1_gen():
                    """Out-projection over g=0..3 (heads 0..7, normalized by
                    the first denominator batch); interleaves into f=7."""
                    for m in range(KC):
                        pa1 = psK.tile([P, NQ], f32, tag="kp", name=f"c1a{m}")
                        pb1 = psNK.tile([P, NQ], f32, tag="nkb", name=f"c1b{m}")
                        for g in range(KC // 2):
                            nc.tensor.matmul(pa1[:], wo_sb[0:64, m, g, :],
                                             onTa[0:64, g, :],
                                             start=(g == 0), stop=(g == 3),
                                             tile_position=(0, 0))
                            nc.tensor.matmul(pb1[:], wo_sb[64:P, m, g, :],
                                             onTa[64:P, g, :],
                                             start=(g == 0), stop=(g == 3),
                                             tile_position=(64, 0))
                        u = pc.tile([P, NQ], f32, tag="c1u")
                        nc.vector.tensor_copy(u[:], pb1[:])
                        nc.vector.scalar_tensor_tensor(
                            out=oc1[:, m, :], in0=pa1[:],
                            scalar=bout_sb[:, m:m + 1], in1=u[:],
                            op0=ALU.add, op1=ALU.add)
                        yield

                # main loop; f=7 interleaves the first half of the
                # output projection instead of a K chunk
                for f in range(KC):
                    if f < KC - 1:
                        g = kproj_gen(f + 1, xt)
                    else:
                        g = phasec1_gen()
                    attn_f(f, g)
                    for _ in g:
                        pass

                # ---- Phase C2: out-projection g=4..7 + combine ----
                for m in range(KC):
                    pa2 = psK.tile([P, NQ], f32, tag="kp", name=f"c2a{m}")
                    pb2 = psNK.tile([P, NQ], f32, tag="nkb", name=f"c2b{m}")
                    for g in range(KC // 2, KC):
                        nc.tensor.matmul(pa2[:], wo_sb[0:64, m, g, :],
                                         onTb[0:64, g - 4, :],
                                         start=(g == 4), stop=(g == KC - 1),
                                         tile_position=(0, 0))
                        nc.tensor.matmul(pb2[:], wo_sb[64:P, m, g, :],
                                         onTb[64:P, g - 4, :],
                                         start=(g == 4), stop=(g == KC - 1),
                                         tile_position=(64, 0))
                    t3 = pc.tile([P, NQ], f32, tag="t3")
                    nc.vector.scalar_tensor_tensor(
                        out=t3[:], in0=pa2[:], scalar=0.0,
                        in1=oc1[:, m, :], op0=ALU.add, op1=ALU.add)
                    ot = pbr.tile([P, NQ], f32, tag="ot")
                    nc.vector.scalar_tensor_tensor(
                        out=ot[:], in0=pb2[:], scalar=0.0,
                        in1=t3[:], op0=ALU.add, op1=ALU.add)
                    nc.sync.dma_start(out=outT[m * P:(m + 1) * P, :], in_=ot[:])

    nc.compile()
    _CACHE["nc"] = nc
    return nc


def _layout(w):
    # [DIM, C] -> [P, KC, C] with row d = kc*128 + p
    c = w.shape[1]
    return np.ascontiguousarray(w.reshape(KC, P, c).transpose(1, 0, 2))


def run(inputs, trace=False):
    import ml_dtypes
    from concourse.bass_utils import run_bass_kernel_spmd

    x = np.asarray(inputs["x"], np.float32)
    w_qkv = np.asarray(inputs["w_qkv"], np.float32)
    w_out = np.asarray(inputs["w_out"], np.float32)
    b_out = np.asarray(inputs["b_out"], np.float32)
    logit_scale = np.asarray(inputs["logit_scale"], np.float32)

    nc = _build()

    bf = ml_dtypes.bfloat16

    def _wtile(w):
        # [DIM, DIM] -> [P, KC(m), KC(kc), P]: tile (kc, m) is w[kc*128+p, m*128+q]
        return np.ascontiguousarray(
            w.reshape(KC, P, KC, P).transpose(1, 2, 0, 3))

    wqb = _wtile(w_qkv[:, 0:INNER]).astype(bf)
    wkb = _wtile(w_qkv[:, INNER:2 * INNER]).astype(bf)
    wvb = np.ascontiguousarray(
        w_qkv[:, 2 * INNER:3 * INNER].reshape(KC, P, 2, INNER // 2)
        .transpose(1, 2, 0, 3)).astype(bf)
    wo2 = _wtile(w_out).astype(bf)
    bout = np.ascontiguousarray(b_out.reshape(KC, P).T)
    scale = np.exp(np.minimum(logit_scale.reshape(H), MAX_LOG_SCALE)).astype(
        np.float32)
    sclb = np.zeros((P, 2), np.float32)
    for h in range(H):
        m, half = h // 2, h % 2
        sclb[32 * (m % 4) + half, m // 4] = scale[h]

    xTb = [(_layout(np.ascontiguousarray(x[b].T)).astype(bf)) for b in range(B)]

    in_maps = []
    for c in range(8):
        b, qs = c // 4, c % 4
        xrot = np.ascontiguousarray(np.roll(xTb[b], -qs * NQ, axis=2))
        in_maps.append({
            "xq": np.ascontiguousarray(xrot[:, :, 0:NQ]),
            "xTb": xrot,
            "wqb": wqb, "wkb": wkb, "wvb": wvb, "wo2": wo2,
            "bout": bout, "sclb": sclb,
        })

    res = run_bass_kernel_spmd(nc, in_maps, list(range(8)), trace=trace)

    out = np.empty((B, N, DIM), np.float32)
    for c in range(8):
        b, qs = c // 4, c % 4
        out[b, qs * NQ:(qs + 1) * NQ, :] = res.results[c]["outT"].T
    return out, res


def kernel(**inputs):
    out, _ = run(inputs, trace=False)
    return out


# revision 43
# speedup vs baseline: 1.2052x; 1.2052x over previous
"""Cosine-similarity multi-head attention on 8 Trainium2 NeuronCores.

Sharding: data/sequence-parallel. Core c (c = b*4 + qs) computes the full
output rows for query tokens [qs*512, (qs+1)*512) of batch b.  Each core
computes K and V for its whole batch (duplicated 4x across the cores sharing
the batch; collectives measured too slow to be worth deduplicating).

Design notes (v3):
  - Whole attention path in bf16 (qnT/knf/et/av/onT/wo).
  - K projection is *lazy*: chunk f (the feature rows of head pair 2f,2f+1)
    is projected + normalized one f ahead, interleaved into phase B's S/AV
    slots, keeping the PE busy while the ACT engine streams softmax exps
    (16.8M elements at 1 elem/cycle = the phase-B floor).  kproj(0) is
    interleaved into the tail of phase A's V projection the same way.
  - All norm rsqrts via a DVE bit-trick (+2 Newton steps); the tiny
    per-block |k|^2 matmuls are col-tiled (tile_position=(0,32t)) into one
    PSUM bank so one [128,512] DVE chain covers a whole head pair.
  - Output projection is split: g=0..3 (heads normalized by the first
    denominator batch) runs interleaved into f=7's slots and is stashed to
    SBUF; g=4..7 runs after the last denominator batch.  This fills the
    phase-B tail and removes the B->C bubble.
  - softmax uses no max-subtraction: |logits| <= scale = 10, exp safe f32.
"""

import numpy as np

B, N, DIM, H, DH = 2, 2048, 1024, 16, 64
INNER = H * DH
NQ = 512            # query tokens per core
P = 128
KC = DIM // P       # 8 feature chunks of 128
JC = N // P         # 16 key-token chunks of 128
NB = N // NQ        # 4 token blocks of 512
MAX_LOG_SCALE = float(np.log(1.0 / 0.01))
MAGIC = 0x5F3759DF

_CACHE = {}


def _build():
    if "nc" in _CACHE:
        return _CACHE["nc"]
    import concourse.bass as bass
    import concourse.bacc as bacc
    import concourse.mybir as mybir
    import concourse.tile as tile

    f32 = mybir.dt.float32
    i32 = mybir.dt.int32
    bf16 = mybir.dt.bfloat16
    AF = mybir.ActivationFunctionType
    ALU = mybir.AluOpType

    nc = bacc.Bacc("TRN2", target_bir_lowering=False)

    xq_in = nc.declare_dram_parameter("xq", [P, KC, NQ], bf16, isOutput=False)
    xTb = nc.declare_dram_parameter("xTb", [P, KC, N], bf16, isOutput=False)
    wqb = nc.declare_dram_parameter("wqb", [P, KC, KC, P], bf16, isOutput=False)
    wkb = nc.declare_dram_parameter("wkb", [P, KC, KC, P], bf16, isOutput=False)
    wvb = nc.declare_dram_parameter("wvb", [P, 2, KC, INNER // 2], bf16, isOutput=False)
    wo2 = nc.declare_dram_parameter("wo2", [P, KC, KC, P], bf16, isOutput=False)
    bout = nc.declare_dram_parameter("bout", [P, KC], f32, isOutput=False)
    # scale_h spread to rows 32*(m%4)+half, col = bank m//4 (norm-matmul layout)
    sclb = nc.declare_dram_parameter("sclb", [P, 2], f32, isOutput=False)
    outT = nc.declare_dram_parameter("outT", [DIM, NQ], f32, isOutput=True)

    # internal DRAM scratch (broadcast round trips)
    fq_d = nc.dram_tensor("fq_d", [2, P, NQ], f32)
    fk_d = nc.dram_tensor("fk_d", [KC, P, NQ], bf16)
    dnf_d = nc.dram_tensor("dnf_d", [H, NQ], f32)
    dnm_d = nc.dram_tensor("dnm_d", [H, NQ], bf16)

    def rsqrt_dve(y, x, u, out=None):
        """y/out = 1/sqrt(x) elementwise on DVE (bit-trick + 2 Newton).
        x,y,u f32 APs (same shape); if out given the final step writes it."""
        nc.vector.tensor_scalar(
            out=y.bitcast(i32), in0=x.bitcast(i32),
            scalar1=1, scalar2=None, op0=ALU.arith_shift_right)
        nc.vector.tensor_scalar(
            out=y.bitcast(i32), in0=y.bitcast(i32),
            scalar1=-1, scalar2=MAGIC, op0=ALU.mult, op1=ALU.add)
        for it in range(2):
            nc.vector.tensor_mul(u, y, y)
            nc.vector.tensor_mul(u, u, x)
            nc.vector.tensor_scalar(
                out=u, in0=u, scalar1=-0.5, scalar2=1.5,
                op0=ALU.mult, op1=ALU.add)
            dst = y if (it == 0 or out is None) else out
            nc.vector.tensor_mul(dst, y, u)

    with tile.TileContext(nc) as tc:
        with (
            tc.tile_pool(name="persist", bufs=1) as pp,
            tc.tile_pool(name="pBk", bufs=2) as pbk,
            tc.tile_pool(name="pBs", bufs=2) as pbs,
            tc.tile_pool(name="psK", bufs=1, space="PSUM") as psK,
            tc.tile_pool(name="psNK", bufs=1, space="PSUM") as psNK,
        ):
            xq = pp.tile([P, KC, NQ], bf16, tag="xq")
            xt = pp.tile([P, KC, N], bf16, tag="xt")
            knf = pp.tile([P, KC, N], bf16, tag="knf")
            av = pp.tile([P, JC, H, DH + 1], bf16, tag="av")
            qnT = pp.tile([P, KC, NQ], bf16, tag="qnT")
            onTa = pp.tile([P, KC // 2, NQ], bf16, tag="onTa")
            onTb = pp.tile([P, KC // 2, NQ], bf16, tag="onTb")
            wo_sb = pp.tile([P, KC, KC, P], bf16, tag="wo")
            scale_sb = pp.tile([P, 2], f32, tag="scale")
            bout_sb = pp.tile([P, KC], f32, tag="bout")
            hsm2 = pp.tile([P, 2], bf16, tag="hsm2")

            def onT(g):
                return onTa[:, g, :] if g < KC // 2 else onTb[:, g - KC // 2, :]

            # queue layout: sync = xq + per-m Q weights + broadcasts;
            # gpsimd = xt + wo; vector = wvb; scalar = small consts
            nc.sync.dma_start(out=xq[:], in_=xq_in[:])
            nc.scalar.dma_start(out=scale_sb[:], in_=sclb[:])
            nc.scalar.dma_start(out=bout_sb[:], in_=bout[:])
            for t in range(NB):
                nc.gpsimd.dma_start(out=xt[:, :, t * NQ:(t + 1) * NQ],
                                    in_=xTb[:, :, t * NQ:(t + 1) * NQ])
            nc.scalar.dma_start(out=wo_sb[:], in_=wo2[:])
            nc.vector.memset(hsm2[:], 0.0)
            nc.vector.memset(hsm2[0:64, 0:1], 1.0)
            nc.vector.memset(hsm2[64:P, 1:2], 1.0)
            nc.vector.memset(av[:, :, :, DH], 1.0)

            def kproj_gen(f, src, wq=None):
                """Project + normalize K chunk f into knf[:, f, :].
                Yields 8x (2 per token block); norm chain emitted at drain."""
                wt = pbk.tile([P, KC, P], bf16, tag="wk", name=f"wk{f}")
                (wq or nc.sync).dma_start(out=wt[:], in_=wkb[:, f])
                nkb = psNK.tile([P, NQ], f32, tag="nkb", name=f"nkb{f}")
                for t in range(NB):
                    tsl = slice(t * NQ, (t + 1) * NQ)
                    ps = psK.tile([P, NQ], f32, tag="kp", name=f"kps{f}_{t}")
                    for kc in range(4):
                        nc.tensor.matmul(ps[:], wt[:, kc, :], src[:, kc, tsl],
                                         start=(kc == 0), stop=False)
                    yield
                    for kc in range(4, KC):
                        nc.tensor.matmul(ps[:], wt[:, kc, :], src[:, kc, tsl],
                                         start=False, stop=(kc == KC - 1))
                    nc.vector.tensor_copy(knf[:, f, tsl], ps[:])
                    sq = pbs.tile([P, NQ], bf16, tag="ksq")
                    nc.gpsimd.tensor_mul(sq[:], knf[:, f, tsl], knf[:, f, tsl])
                    co = 32 * t
                    nc.tensor.matmul(nkb[co:co + 2, :], hsm2[:], sq[:],
                                     start=True, stop=True,
                                     tile_position=(0, co))
                    yield
                # 1/|k| and fold into knf; rows 32t+half of the norm bank
                # carry (block t, head 2f+half), other rows are junk
                fkx = pbs.tile([P, NQ], f32, tag="fkx")
                nc.vector.tensor_copy(fkx[:], nkb[:])
                fku = pbs.tile([P, NQ], f32, tag="fku")
                fkv = pbs.tile([P, NQ], f32, tag="fkv")
                fk16 = pbs.tile([P, NQ], bf16, tag="fk16")
                rsqrt_dve(fku[:], fkx[:], fkv[:], out=fk16[:])
                nc.sync.dma_start(out=fk_d[f], in_=fk16[:])
                for t in range(NB):
                    tsl = slice(t * NQ, (t + 1) * NQ)
                    ro = 32 * t
                    fkb = pbs.tile([P, NQ], bf16, tag="fkb")
                    nc.sync.dma_start(
                        out=fkb[0:64, :],
                        in_=fk_d[f, ro:ro + 1, :].to_broadcast((64, NQ)))
                    nc.sync.dma_start(
                        out=fkb[64:P, :],
                        in_=fk_d[f, ro + 1:ro + 2, :].to_broadcast((64, NQ)))
                    nc.vector.tensor_mul(knf[:, f, tsl], knf[:, f, tsl], fkb[:])

            # ---------------- Phase A: Q + V projections ----------------
            with (
                tc.tile_pool(name="pA", bufs=2) as pa,
                tc.tile_pool(name="pAv", bufs=1) as pav,
                tc.tile_pool(name="pAs", bufs=2) as pas,
                tc.tile_pool(name="pAq", bufs=1) as paq,
                tc.tile_pool(name="psA", bufs=2, space="PSUM") as psA,
                tc.tile_pool(name="psN", bufs=2, space="PSUM") as psN,
            ):
                qraw = paq.tile([P, KC, NQ], f32, tag="qraw")

                nqb = [psN.tile([P, NQ], f32, tag="nqb", name=f"nqb{bk}")
                       for bk in range(2)]
                for m in range(KC):
                    wt = pa.tile([P, KC, P], bf16, tag="w")
                    nc.sync.dma_start(out=wt[:], in_=wqb[:, m])
                    ps = psA.tile([P, NQ], f32, tag="qp")
                    for kc in range(KC):
                        nc.tensor.matmul(ps[:], wt[:, kc, :], xq[:, kc, :],
                                         start=(kc == 0), stop=(kc == KC - 1))
                    nc.scalar.copy(qraw[:, m, :], ps[:])
                    sq = pas.tile([P, NQ], bf16, tag="sq")
                    nc.vector.tensor_mul(sq[:], qraw[:, m, :], qraw[:, m, :])
                    co = 32 * (m % 4)
                    nc.tensor.matmul(nqb[m // 4][co:co + 2, :], hsm2[:], sq[:],
                                     start=True, stop=True,
                                     tile_position=(0, co))

                # fq = scale_h / |q|  (rows 32*(m%4)+half carry head 2m+half)
                for bk in range(2):
                    fqx = pas.tile([P, NQ], f32, tag="fqx")
                    nc.vector.tensor_copy(fqx[:], nqb[bk][:])
                    fqy = pas.tile([P, NQ], f32, tag="fqy")
                    fqu = pas.tile([P, NQ], f32, tag="fqu")
                    rsqrt_dve(fqy[:], fqx[:], fqu[:])
                    nc.vector.tensor_scalar_mul(fqy[:], fqy[:],
                                                scale_sb[:, bk:bk + 1])
                    nc.sync.dma_start(out=fq_d[bk], in_=fqy[:])
                for m in range(KC):
                    ro = 32 * (m % 4)
                    fqb = pas.tile([P, NQ], f32, tag="fqb")
                    nc.sync.dma_start(
                        out=fqb[0:64, :],
                        in_=fq_d[m // 4, ro:ro + 1, :].to_broadcast((64, NQ)))
                    nc.sync.dma_start(
                        out=fqb[64:P, :],
                        in_=fq_d[m // 4, ro + 1:ro + 2, :].to_broadcast((64, NQ)))
                    nc.vector.tensor_mul(qnT[:, m, :], qraw[:, m, :], fqb[:])

                # V projection (token-major into av, bf16), with K chunk 0
                # interleaved into the second half
                k0 = kproj_gen(0, xt)
                for fb in range(2):
                    wv = pav.tile([P, KC, INNER // 2], bf16, tag="wv")
                    nc.gpsimd.dma_start(out=wv[:], in_=wvb[:, fb])
                    for jc in range(JC):
                        ps = psA.tile([P, NQ], f32, tag="vp")
                        for kc in range(KC):
                            nc.tensor.matmul(ps[:], xt[:, kc, jc * P:(jc + 1) * P],
                                             wv[:, kc, :],
                                             start=(kc == 0), stop=(kc == KC - 1))
                        nc.vector.tensor_copy(
                            av[:, jc, fb * 8:(fb + 1) * 8, 0:DH],
                            ps[:].rearrange("p (h d) -> p h d", d=DH))
                        if fb == 1:
                            if 4 <= jc < 8:
                                next(k0, None)
                                next(k0, None)
                            elif jc == 8:
                                for _ in k0:
                                    pass

            # ---------------- Phase B: lazy K proj + attention ----------------
            with (
                tc.tile_pool(name="pBe", bufs=3) as pbe,
                tc.tile_pool(name="pBr", bufs=2) as pbr,
                tc.tile_pool(name="pC", bufs=1) as pc,
                tc.tile_pool(name="pC1", bufs=1) as pc1,
                tc.tile_pool(name="psS", bufs=2, space="PSUM") as psS,
                tc.tile_pool(name="psV", bufs=1, space="PSUM") as psV,
            ):
                oc1 = pc1.tile([P, KC, NQ], f32, tag="oc1")

                def emit_av(f, jc2, ets, avps):
                    for q in range(2):
                        jc = 2 * jc2 + q
                        for half in range(2):
                            h = 2 * f + half
                            nc.tensor.matmul(
                                avps[half][0:DH + 1, :],
                                av[:, jc, h, :], ets[(jc2, half)][:, q, :],
                                start=(jc == 0), stop=(jc == JC - 1))

                def attn_f(f, ksteps):
                    avps = [psV.tile([P, NQ], f32, tag=f"avp{half}",
                                     name=f"avp{half}_{f}")
                            for half in range(2)]
                    ets = {}
                    for jc2 in range(8):
                        sp2 = []
                        for half in range(2):
                            sp2.append(psS.tile([P, 2, NQ], f32,
                                                tag="sps",
                                                name=f"sps{half}_{f}_{jc2}"))
                        for q in range(2):
                            jc = 2 * jc2 + q
                            for half in range(2):
                                lo = 64 * half
                                nc.tensor.matmul(
                                    sp2[half][:, q, :],
                                    knf[lo:lo + 64, f, jc * P:(jc + 1) * P],
                                    qnT[lo:lo + 64, f, :],
                                    start=True, stop=True, tile_position=(lo, 0))
                        for half in range(2):
                            et = pbe.tile([P, 2, NQ], bf16, tag=f"et{half}")
                            nc.scalar.activation(et[:], sp2[half][:], AF.Exp)
                            ets[(jc2, half)] = et
                        if ksteps is not None:
                            if jc2 < 4:
                                next(ksteps, None)
                                next(ksteps, None)
                            elif jc2 == 4:
                                for _ in ksteps:
                                    pass
                        if jc2 > 0:
                            emit_av(f, jc2 - 1, ets, avps)
                    emit_av(f, 7, ets, avps)
                    # attention outputs + denominators for this head pair
                    for half in range(2):
                        h = 2 * f + half
                        lo = 64 * half
                        nc.vector.tensor_copy(onT(f)[lo:lo + 64, :],
                                              avps[half][0:DH, :])
                        dcp = pbr.tile([1, NQ], f32, tag="dcp")
                        nc.vector.tensor_copy(dcp[:], avps[half][DH:DH + 1, :])
                        nc.sync.dma_start(out=dnf_d[h:h + 1, :], in_=dcp[:])
                    # normalize finished head pairs in two batches
                    if f in (KC // 2 - 1, KC - 1):
                        hlo = 0 if f == KC // 2 - 1 else H // 2
                        dnm = pbr.tile([H // 2, NQ], f32, tag="dnm")
                        nc.sync.dma_start(out=dnm[:], in_=dnf_d[hlo:hlo + 8, :])
                        nc.vector.reciprocal(dnm[:], dnm[:])
                        dnr = pbr.tile([H // 2, NQ], bf16, tag="dnr")
                        nc.vector.tensor_copy(dnr[:], dnm[:])
                        nc.sync.dma_start(out=dnm_d[hlo:hlo + 8, :], in_=dnr[:])
                        for ff in range(hlo // 2, hlo // 2 + 4):
                            g = ff if hlo == 0 else ff
                            dnb = pbr.tile([P, NQ], bf16, tag="dnb")
                            nc.sync.dma_start(
                                out=dnb[0:64, :],
                                in_=dnm_d[2 * ff:2 * ff + 1, :]
                                .to_broadcast((64, NQ)))
                            nc.sync.dma_start(
                                out=dnb[64:P, :],
                                in_=dnm_d[2 * ff + 1:2 * ff + 2, :]
                                .to_broadcast((64, NQ)))
                            nc.vector.tensor_mul(onT(ff), onT(ff), dnb[:])

                def phasec1_gen():
                    """Out-projection over g=0..3 (heads 0..7, normalized by
                    the first denominator batch); interleaves into f=7."""
                    for m in range(KC):
                        pa1 = psK.tile([P, NQ], f32, tag="kp", name=f"c1a{m}")
                        pb1 = psNK.tile([P, NQ], f32, tag="nkb", name=f"c1b{m}")
                        for g in range(KC // 2):
                            nc.tensor.matmul(pa1[:], wo_sb[0:64, m, g, :],
                                             onTa[0:64, g, :],
                                             start=(g == 0), stop=(g == 3),
                                             tile_position=(0, 0))
                            nc.tensor.matmul(pb1[:], wo_sb[64:P, m, g, :],
                                             onTa[64:P, g, :],
                                             start=(g == 0), stop=(g == 3),
                                             tile_position=(64, 0))
                        u = pc.tile([P, NQ], f32, tag="c1u")
                        nc.vector.tensor_copy(u[:], pb1[:])
                        nc.vector.scalar_tensor_tensor(
                            out=oc1[:, m, :], in0=pa1[:],
                            scalar=bout_sb[:, m:m + 1], in1=u[:],
                            op0=ALU.add, op1=ALU.add)
                        yield

                # main loop; f=7 interleaves the first half of the
                # output projection instead of a K chunk
                for f in range(KC):
                    if f < KC - 1:
                        g = kproj_gen(f + 1, xt)
                    else:
                        g = phasec1_gen()
                    attn_f(f, g)
                    for _ in g:
                        pass

                # ---- Phase C2: out-projection g=4..7 + combine ----
                for m in range(KC):
                    pa2 = psK.tile([P, NQ], f32, tag="kp", name=f"c2a{m}")
                    pb2 = psNK.tile([P, NQ], f32, tag="nkb", name=f"c2b{m}")
                    for g in range(KC // 2, KC):
                        nc.tensor.matmul(pa2[:], wo_sb[0:64, m, g, :],
                                         onTb[0:64, g - 4, :],
                                         start=(g == 4), stop=(g == KC - 1),
                                         tile_position=(0, 0))
                        nc.tensor.matmul(pb2[:], wo_sb[64:P, m, g, :],
                                         onTb[64:P, g - 4, :],
                                         start=(g == 4), stop=(g == KC - 1),
                                         tile_position=(64, 0))
                    t3 = pc.tile([P, NQ], f32, tag="t3")
                    nc.vector.scalar_tensor_tensor(
                        out=t3[:], in0=pa2[:], scalar=0.0,
                        in1=oc1[:, m, :], op0=ALU.add, op1=ALU.add)
                    ot = pbr.tile([P, NQ], f32, tag="ot")
                    nc.vector.scalar_tensor_tensor(
                        out=ot[:], in0=pb2[:], scalar=0.0,
                        in1=t3[:], op0=ALU.add, op1=ALU.add)
                    nc.sync.dma_start(out=outT[m * P:(m + 1) * P, :], in_=ot[:])

    nc.compile()
    _CACHE["nc"] = nc
    return nc


def _layout(w):
    # [DIM, C] -> [P, KC, C] with row d = kc*128 + p
    c = w.shape[1]
    return np.ascontiguousarray(w.reshape(KC, P, c).transpose(1, 0, 2))


def run(inputs, trace=False):
    import ml_dtypes
    from concourse.bass_utils import run_bass_kernel_spmd

    x = np.asarray(inputs["x"], np.float32)
    w_qkv = np.asarray(inputs["w_qkv"], np.float32)
    w_out = np.asarray(inputs["w_out"], np.float32)
    b_out = np.asarray(inputs["b_out"], np.float32)
    logit_scale = np.asarray(inputs["logit_scale"], np.float32)

    nc = _build()

    bf = ml_dtypes.bfloat16

    def _wtile(w):
        # [DIM, DIM] -> [P, KC(m), KC(kc), P]: tile (kc, m) is w[kc*128+p, m*128+q]
        return np.ascontiguousarray(
            w.reshape(KC, P, KC, P).transpose(1, 2, 0, 3))

    wqb = _wtile(w_qkv[:, 0:INNER]).astype(bf)
    wkb = _wtile(w_qkv[:, INNER:2 * INNER]).astype(bf)
    wvb = np.ascontiguousarray(
        w_qkv[:, 2 * INNER:3 * INNER].reshape(KC, P, 2, INNER // 2)
        .transpose(1, 2, 0, 3)).astype(bf)
    wo2 = _wtile(w_out).astype(bf)
    bout = np.ascontiguousarray(b_out.reshape(KC, P).T)
    scale = np.exp(np.minimum(logit_scale.reshape(H), MAX_LOG_SCALE)).astype(
        np.float32)
    sclb = np.zeros((P, 2), np.float32)
    for h in range(H):
        m, half = h // 2, h % 2
        sclb[32 * (m % 4) + half, m // 4] = scale[h]

    xTb = [(_layout(np.ascontiguousarray(x[b].T)).astype(bf)) for b in range(B)]

    in_maps = []
    for c in range(8):
        b, qs = c // 4, c % 4
        xrot = np.ascontiguousarray(np.roll(xTb[b], -qs * NQ, axis=2))
        in_maps.append({
            "xq": np.ascontiguousarray(xrot[:, :, 0:NQ]),
            "xTb": xrot,
            "wqb": wqb, "wkb": wkb, "wvb": wvb, "wo2": wo2,
            "bout": bout, "sclb": sclb,
        })

    res = run_bass_kernel_spmd(nc, in_maps, list(range(8)), trace=trace)

    out = np.empty((B, N, DIM), np.float32)
    for c in range(8):
        b, qs = c // 4, c % 4
        out[b, qs * NQ:(qs + 1) * NQ, :] = res.results[c]["outT"].T
    return out, res


def kernel(**inputs):
    out, _ = run(inputs, trace=False)
    return out
